# revision 16
# baseline (speedup 1.0000x reference)
"""ChromDropout kernel for one TRN2 chip (8 NeuronCores, data-parallel).

Math (training-mode ChromDropout):
    out[b, g] = x[b, g] * (1 - drop[b, chrom_ids[g]]) * (NUM_CHROMS / N_DROP)
where drop[b, :] marks 4 distinct chromosomes sampled per row with
jax.random.permutation(split(key(42), B)[b], 23)[:4].

Strategy:
  - Host (tiny): derive the per-row keep table exactly as the reference does
    (threefry is platform-deterministic) and a [23, G] one-hot of chrom_ids.
    Both are {0,1}-valued -> shipped as fp8 (exact), 21KB/partition in SBUF.
  - Device (all the heavy traffic): shard x row-wise across 8 cores
    (2048 rows each), processed as 16 tiles of 128 adjacent rows x full
    19064-gene width — each tile load/store is one fully contiguous 9.75MB
    DMA. Per tile, mask01[128, N] = keepT[23, 128].T @ onehot[23, N] on the
    TensorEngine (exact one-hot selection), then a single fused VectorEngine
    op out = (x * 5.75) * mask01 per chunk. Streaming, memory-bound.

The 5.75 rescale placement keeps bitwise equality with the reference:
(x*1.0)*5.75 and (x*5.75)*1.0 round identically; dropped genes give +-0.0
either way.
"""

import numpy as np

B = 16384
G = 19064
C = 23
ND = 4
SCALE = float(C) / float(ND)  # 5.75
NCORES = 8
BS = B // NCORES  # 2048 rows per core
P = 128
NT = BS // P  # 16 row groups of 128 adjacent rows
XBUFS = 2  # double-buffered full-width tiles (152KB/partition + 21KB tables)
MM = 512  # matmul moving-free-dim chunk (one PSUM bank of f32)

_CACHED = {}


def _build_nc():
    import concourse.bacc as bacc
    import concourse.mybir as mybir
    from concourse.tile import TileContext

    f32 = mybir.dt.float32
    fp8 = mybir.dt.float8e4
    # Bacc (not raw Bass): its compile() runs move_matmul_waits_to_ldweights +
    # generate_event_semaphores, which split multi-semaphore waits down to the
    # 1-wait-per-instruction TRN2 ISA limit.
    nc = bacc.Bacc("TRN2", target_bir_lowering=False, debug=False)
    x = nc.declare_dram_parameter("x", [BS, G], f32, isOutput=False)
    # columns [0, BS) = per-row keep indicator (transposed, row-order columns),
    # [BS, BS+G) = gene one-hot. One parameter -> one DMA -> one wait
    # semaphore for every matmul.
    tables = nc.declare_dram_parameter("tables", [C, BS + G], fp8, isOutput=False)
    out = nc.declare_dram_parameter("out", [BS, G], f32, isOutput=True)

    x_t = x[:, :].rearrange("(t p) g -> t p g", p=P)
    out_t = out[:, :].rearrange("(t p) g -> t p g", p=P)

    with TileContext(nc) as tc:
        with (
            tc.tile_pool(name="const", bufs=1) as const_pool,
            tc.tile_pool(name="xp", bufs=XBUFS) as xp,
            tc.tile_pool(name="pp", bufs=8, space="PSUM") as pp,
        ):
            tbl = const_pool.tile([C, BS + G], fp8, tag="tbl")
            nc.sync.dma_start(tbl[:], tables[:])

            # quarter boundaries aligned to MM chunks (used for the last tile)
            QB = [0, 10 * MM, 20 * MM, 30 * MM, G]

            for t in range(NT):  # 16 row groups of 128 adjacent rows
                last = t == NT - 1
                xt = xp.tile([P, G], f32, tag="xt")
                if last:
                    # Split the last tile's load so DVE (and then the store
                    # quarters) start ~18us earlier: Tile tracks sub-tile
                    # ranges, so each DVE chunk waits only on its quarter.
                    for a, b in zip(QB[:-1], QB[1:]):
                        nc.sync.dma_start(xt[:, a:b], x_t[t, :, a:b])
                else:
                    nc.sync.dma_start(xt[:], x_t[t])
                for off in range(0, G, MM):
                    w = min(MM, G - off)
                    m = pp.tile([P, MM], f32, tag="m")
                    nc.tensor.matmul(
                        m[:, :w],
                        tbl[:, t * P : (t + 1) * P],
                        tbl[:, BS + off : BS + off + w],
                        start=True,
                        stop=True,
                    )
                    # out = (x * 5.75) * mask01, single DVE op
                    nc.vector.scalar_tensor_tensor(
                        xt[:, off : off + w],
                        xt[:, off : off + w],
                        SCALE,
                        m[:, :w],
                        mybir.AluOpType.mult,
                        mybir.AluOpType.mult,
                    )
                if last:
                    for a, b in zip(QB[:-1], QB[1:]):
                        nc.sync.dma_start(out_t[t, :, a:b], xt[:, a:b])
                else:
                    nc.sync.dma_start(out_t[t], xt[:])
    nc.finalize()  # Bacc.finalize -> compile() (wait splitting etc) + freeze
    return nc


def _host_tables(chrom_ids: np.ndarray) -> tuple[np.ndarray, np.ndarray]:
    """keep [B, 23] in {0,1}; onehot [23, G] in {0,1} - both float32."""
    import jax

    with jax.default_device(jax.devices("cpu")[0]):
        keys = jax.random.split(jax.random.key(42), B)
        sel = np.asarray(
            jax.vmap(lambda k: jax.random.permutation(k, C)[:ND])(keys)
        )  # [B, 4] int32
    keep = np.ones((B, C), np.float32)
    keep[np.arange(B)[:, None], sel] = 0.0
    onehot = (
        np.asarray(chrom_ids)[None, :] == np.arange(C, dtype=np.int32)[:, None]
    ).astype(np.float32)  # [23, G]
    return keep, onehot


def kernel(x: np.ndarray, chrom_ids: np.ndarray, **run_kwargs) -> np.ndarray:
    import ml_dtypes

    from concourse.bass_utils import run_bass_kernel_spmd

    x = np.asarray(x)
    keep, onehot = _host_tables(chrom_ids)
    fp8 = ml_dtypes.float8_e4m3
    onehot_8 = onehot.astype(fp8)

    if "nc" not in _CACHED:
        _CACHED["nc"] = _build_nc()
    nc = _CACHED["nc"]

    in_maps = []
    for i in range(NCORES):
        keep_t = keep[i * BS : (i + 1) * BS].T  # [23, 2048], row-order columns
        in_maps.append(
            {
                "x": np.ascontiguousarray(x[i * BS : (i + 1) * BS]),
                "tables": np.ascontiguousarray(
                    np.concatenate([keep_t.astype(fp8), onehot_8], axis=1)
                ),
            }
        )
    res = run_bass_kernel_spmd(nc, in_maps, core_ids=list(range(NCORES)), **run_kwargs)
    out = np.concatenate([np.asarray(r["out"]) for r in res.results], axis=0)
    if res.exec_time_ns is not None:
        kernel.last_exec_time_ns = res.exec_time_ns
    kernel.last_results = res
    return out


# revision 17
# speedup vs baseline: 1.0283x; 1.0283x over previous
"""ChromDropout kernel for one TRN2 chip (8 NeuronCores, data-parallel).

Math (training-mode ChromDropout):
    out[b, g] = x[b, g] * (1 - drop[b, chrom_ids[g]]) * (NUM_CHROMS / N_DROP)
where drop[b, :] marks 4 distinct chromosomes sampled per row with
jax.random.permutation(split(key(42), B)[b], 23)[:4].

Strategy:
  - Host (tiny): derive the per-row keep table exactly as the reference does
    (threefry is platform-deterministic) and a [23, G] one-hot of chrom_ids.
    Both are {0,1}-valued -> shipped as fp8 (exact), 21KB/partition in SBUF.
  - Device (all the heavy traffic): shard x row-wise across 8 cores
    (2048 rows each), processed as 16 tiles of 128 adjacent rows x full
    19064-gene width — each tile load/store is one fully contiguous 9.75MB
    DMA. Per tile, mask01[128, N] = keepT[23, 128].T @ onehot[23, N] on the
    TensorEngine (exact one-hot selection), then a single fused VectorEngine
    op out = (x * 5.75) * mask01 per chunk. Streaming, memory-bound.

The 5.75 rescale placement keeps bitwise equality with the reference:
(x*1.0)*5.75 and (x*5.75)*1.0 round identically; dropped genes give +-0.0
either way.
"""

import numpy as np

B = 16384
G = 19064
C = 23
ND = 4
SCALE = float(C) / float(ND)  # 5.75
NCORES = 8
BS = B // NCORES  # 2048 rows per core
P = 128
NT = BS // P  # 16 row groups of 128 adjacent rows
XBUFS = 2  # double-buffered full-width tiles (152KB/partition + 21KB tables)
MM = 512  # matmul moving-free-dim chunk (one PSUM bank of f32)

_CACHED = {}


def _build_nc():
    import concourse.bacc as bacc
    import concourse.mybir as mybir
    from concourse.tile import TileContext

    f32 = mybir.dt.float32
    fp8 = mybir.dt.float8e4
    # Bacc (not raw Bass): its compile() runs move_matmul_waits_to_ldweights +
    # generate_event_semaphores, which split multi-semaphore waits down to the
    # 1-wait-per-instruction TRN2 ISA limit.
    nc = bacc.Bacc("TRN2", target_bir_lowering=False, debug=False)
    x = nc.declare_dram_parameter("x", [BS, G], f32, isOutput=False)
    # columns [0, BS) = per-row keep indicator (transposed, row-order columns),
    # [BS, BS+G) = gene one-hot. One parameter -> one DMA -> one wait
    # semaphore for every matmul.
    tables = nc.declare_dram_parameter("tables", [C, BS + G], fp8, isOutput=False)
    out = nc.declare_dram_parameter("out", [BS, G], f32, isOutput=True)

    x_t = x[:, :].rearrange("(t p) g -> t p g", p=P)
    out_t = out[:, :].rearrange("(t p) g -> t p g", p=P)

    with TileContext(nc) as tc:
        with (
            tc.tile_pool(name="const", bufs=1) as const_pool,
            tc.tile_pool(name="xp", bufs=XBUFS) as xp,
            tc.tile_pool(name="pp", bufs=4, space="PSUM") as pp,
        ):
            tbl = const_pool.tile([C, BS + G], fp8, tag="tbl")
            nc.sync.dma_start(tbl[:], tables[:])

            # quarter boundaries aligned to MM chunks (used for the last tile)
            QB = [0, 10 * MM, 20 * MM, 30 * MM, G]

            for t in range(NT):  # 16 row groups of 128 adjacent rows
                last = t == NT - 1
                xt = xp.tile([P, G], f32, tag="xt")
                if last:
                    # Split the last tile's load so DVE (and then the store
                    # quarters) start ~18us earlier: Tile tracks sub-tile
                    # ranges, so each DVE chunk waits only on its quarter.
                    for a, b in zip(QB[:-1], QB[1:]):
                        nc.sync.dma_start(xt[:, a:b], x_t[t, :, a:b])
                else:
                    nc.sync.dma_start(xt[:], x_t[t])
                for off in range(0, G, 2 * MM):
                    # two matmuls fill a 2-bank PSUM tile; one DVE op drains
                    # both (halves DVE op count -> more slack vs the ring)
                    w2 = min(2 * MM, G - off)
                    m = pp.tile([P, 2 * MM], f32, tag="m")
                    for k in range(0, w2, MM):
                        w = min(MM, w2 - k)
                        nc.tensor.matmul(
                            m[:, k : k + w],
                            tbl[:, t * P : (t + 1) * P],
                            tbl[:, BS + off + k : BS + off + k + w],
                            start=True,
                            stop=True,
                        )
                    # out = (x * 5.75) * mask01, single DVE op per 2 banks
                    nc.vector.scalar_tensor_tensor(
                        xt[:, off : off + w2],
                        xt[:, off : off + w2],
                        SCALE,
                        m[:, :w2],
                        mybir.AluOpType.mult,
                        mybir.AluOpType.mult,
                    )
                if last:
                    for a, b in zip(QB[:-1], QB[1:]):
                        nc.sync.dma_start(out_t[t, :, a:b], xt[:, a:b])
                else:
                    nc.sync.dma_start(out_t[t], xt[:])
    nc.finalize()  # Bacc.finalize -> compile() (wait splitting etc) + freeze
    return nc


def _host_tables(chrom_ids: np.ndarray) -> tuple[np.ndarray, np.ndarray]:
    """keep [B, 23] in {0,1}; onehot [23, G] in {0,1} - both float32."""
    import jax

    with jax.default_device(jax.devices("cpu")[0]):
        keys = jax.random.split(jax.random.key(42), B)
        sel = np.asarray(
            jax.vmap(lambda k: jax.random.permutation(k, C)[:ND])(keys)
        )  # [B, 4] int32
    keep = np.ones((B, C), np.float32)
    keep[np.arange(B)[:, None], sel] = 0.0
    onehot = (
        np.asarray(chrom_ids)[None, :] == np.arange(C, dtype=np.int32)[:, None]
    ).astype(np.float32)  # [23, G]
    return keep, onehot


def kernel(x: np.ndarray, chrom_ids: np.ndarray, **run_kwargs) -> np.ndarray:
    import ml_dtypes

    from concourse.bass_utils import run_bass_kernel_spmd

    x = np.asarray(x)
    keep, onehot = _host_tables(chrom_ids)
    fp8 = ml_dtypes.float8_e4m3
    onehot_8 = onehot.astype(fp8)

    if "nc" not in _CACHED:
        _CACHED["nc"] = _build_nc()
    nc = _CACHED["nc"]

    in_maps = []
    for i in range(NCORES):
        keep_t = keep[i * BS : (i + 1) * BS].T  # [23, 2048], row-order columns
        in_maps.append(
            {
                "x": np.ascontiguousarray(x[i * BS : (i + 1) * BS]),
                "tables": np.ascontiguousarray(
                    np.concatenate([keep_t.astype(fp8), onehot_8], axis=1)
                ),
            }
        )
    res = run_bass_kernel_spmd(nc, in_maps, core_ids=list(range(NCORES)), **run_kwargs)
    out = np.concatenate([np.asarray(r["out"]) for r in res.results], axis=0)
    if res.exec_time_ns is not None:
        kernel.last_exec_time_ns = res.exec_time_ns
    kernel.last_results = res
    return out


# revision 20
# speedup vs baseline: 1.1482x; 1.1166x over previous
"""ChromDropout kernel for one TRN2 chip (8 NeuronCores, data-parallel).

Math (training-mode ChromDropout):
    out[b, g] = x[b, g] * (1 - drop[b, chrom_ids[g]]) * (NUM_CHROMS / N_DROP)
where drop[b, :] marks 4 distinct chromosomes sampled per row with
jax.random.permutation(split(key(42), B)[b], 23)[:4].

Strategy:
  - Host (tiny): derive the per-row keep table exactly as the reference does
    (threefry is platform-deterministic) and a [23, G] one-hot of chrom_ids.
    Both are {0,1}-valued -> shipped as fp8 (exact), 21KB/partition in SBUF.
  - Device (all the heavy traffic): shard x row-wise across 8 cores
    (2048 rows each), processed as 16 tiles of 128 adjacent rows x full
    19064-gene width — each tile load/store is one fully contiguous 9.75MB
    DMA. Per tile, mask01[128, N] = keepT[23, 128].T @ onehot[23, N] on the
    TensorEngine (exact one-hot selection), then a single fused VectorEngine
    op out = (x * 5.75) * mask01 per chunk. Streaming, memory-bound.

The 5.75 rescale placement keeps bitwise equality with the reference:
(x*1.0)*5.75 and (x*5.75)*1.0 round identically; dropped genes give +-0.0
either way.
"""

import numpy as np

B = 16384
G = 19064
C = 23
ND = 4
SCALE = float(C) / float(ND)  # 5.75
NCORES = 8
BS = B // NCORES  # 2048 rows per core
P = 128
NT = BS // P  # 16 row groups of 128 adjacent rows
XBUFS = 2  # double-buffered full-width tiles (152KB/partition + 21KB tables)
MM = 512  # matmul moving-free-dim chunk (one PSUM bank of f32)

_CACHED = {}


def _build_nc():
    import concourse.bacc as bacc
    import concourse.mybir as mybir
    from concourse.tile import TileContext

    f32 = mybir.dt.float32
    fp8 = mybir.dt.float8e4
    # Bacc (not raw Bass): its compile() runs move_matmul_waits_to_ldweights +
    # generate_event_semaphores, which split multi-semaphore waits down to the
    # 1-wait-per-instruction TRN2 ISA limit.
    nc = bacc.Bacc("TRN2", target_bir_lowering=False, debug=False)
    x = nc.declare_dram_parameter("x", [BS, G], f32, isOutput=False)
    # columns [0, BS) = per-row keep indicator (transposed, row-order columns),
    # [BS, BS+G) = gene one-hot. One parameter -> one DMA -> one wait
    # semaphore for every matmul.
    tables = nc.declare_dram_parameter("tables", [C, BS + G], fp8, isOutput=False)
    out = nc.declare_dram_parameter("out", [BS, G], f32, isOutput=True)

    x_t = x[:, :].rearrange("(t p) g -> t p g", p=P)
    out_t = out[:, :].rearrange("(t p) g -> t p g", p=P)

    with TileContext(nc) as tc:
        with (
            tc.tile_pool(name="const", bufs=1) as const_pool,
            tc.tile_pool(name="xp", bufs=XBUFS) as xp,
            tc.tile_pool(name="pp", bufs=8, space="PSUM") as pp,
        ):
            tbl = const_pool.tile([C, BS + G], fp8, tag="tbl")
            nc.sync.dma_start(tbl[:], tables[:])

            # quarter boundaries aligned to MM chunks (used for the last tile)
            QB = [0, 10 * MM, 20 * MM, 30 * MM, G]

            for t in range(NT):  # 16 row groups of 128 adjacent rows
                last = t == NT - 1
                xt = xp.tile([P, G], f32, tag="xt")
                if last:
                    # Split the last tile's load so DVE (and then the store
                    # quarters) start ~18us earlier: Tile tracks sub-tile
                    # ranges, so each DVE chunk waits only on its quarter.
                    for a, b in zip(QB[:-1], QB[1:]):
                        nc.sync.dma_start(xt[:, a:b], x_t[t, :, a:b])
                else:
                    nc.sync.dma_start(xt[:], x_t[t])
                for off in range(0, G, MM):
                    w = min(MM, G - off)
                    m = pp.tile([P, MM], f32, tag="m")
                    nc.tensor.matmul(
                        m[:, :w],
                        tbl[:, t * P : (t + 1) * P],
                        tbl[:, BS + off : BS + off + w],
                        start=True,
                        stop=True,
                    )
                    # out = (x * 5.75) * mask01, single DVE op
                    nc.vector.scalar_tensor_tensor(
                        xt[:, off : off + w],
                        xt[:, off : off + w],
                        SCALE,
                        m[:, :w],
                        mybir.AluOpType.mult,
                        mybir.AluOpType.mult,
                    )
                if last:
                    for a, b in zip(QB[:-1], QB[1:]):
                        nc.sync.dma_start(out_t[t, :, a:b], xt[:, a:b])
                else:
                    nc.sync.dma_start(out_t[t], xt[:])
    nc.finalize()  # Bacc.finalize -> compile() (wait splitting etc) + freeze
    return nc


def _host_tables(chrom_ids: np.ndarray) -> tuple[np.ndarray, np.ndarray]:
    """keep [B, 23] in {0,1}; onehot [23, G] in {0,1} - both float32."""
    import jax

    with jax.default_device(jax.devices("cpu")[0]):
        keys = jax.random.split(jax.random.key(42), B)
        sel = np.asarray(
            jax.vmap(lambda k: jax.random.permutation(k, C)[:ND])(keys)
        )  # [B, 4] int32
    keep = np.ones((B, C), np.float32)
    keep[np.arange(B)[:, None], sel] = 0.0
    onehot = (
        np.asarray(chrom_ids)[None, :] == np.arange(C, dtype=np.int32)[:, None]
    ).astype(np.float32)  # [23, G]
    return keep, onehot


def kernel(x: np.ndarray, chrom_ids: np.ndarray, **run_kwargs) -> np.ndarray:
    import ml_dtypes

    from concourse.bass_utils import run_bass_kernel_spmd

    x = np.asarray(x)
    keep, onehot = _host_tables(chrom_ids)
    fp8 = ml_dtypes.float8_e4m3
    onehot_8 = onehot.astype(fp8)

    if "nc" not in _CACHED:
        _CACHED["nc"] = _build_nc()
    nc = _CACHED["nc"]

    in_maps = []
    for i in range(NCORES):
        keep_t = keep[i * BS : (i + 1) * BS].T  # [23, 2048], row-order columns
        in_maps.append(
            {
                "x": np.ascontiguousarray(x[i * BS : (i + 1) * BS]),
                "tables": np.ascontiguousarray(
                    np.concatenate([keep_t.astype(fp8), onehot_8], axis=1)
                ),
            }
        )
    res = run_bass_kernel_spmd(nc, in_maps, core_ids=list(range(NCORES)), **run_kwargs)
    out = np.concatenate([np.asarray(r["out"]) for r in res.results], axis=0)
    if res.exec_time_ns is not None:
        kernel.last_exec_time_ns = res.exec_time_ns
    kernel.last_results = res
    return out
